# revision 24
# baseline (speedup 1.0000x reference)
"""DreamFit single-stream processor block on 8 Trainium2 NeuronCores.

Sharding: 3 heads per core for SDPA (column-parallel linear1), 1/8 of the MLP
per core, row-parallel linear2 (host sums the 8 partial outputs).

Host-side folds (all cheap numpy): LoRA branches into w1/w2 (lora_weight==1),
the modulation (shift/scale/gate from vec) into w1 columns / b1 / w2 rows, so
the device kernel is just: LN-normalize -> linear1 -> qknorm/rope -> SDPA ->
gelu -> linear2 partial. x is passed pre-transposed (and pre-squared, for the
LN stats) so no on-device transposes of x are needed; LN stats come from PE
column-sum matmuls. All matmul operands are bf16 (full PE rate, fast weight
load, half the DMA); accumulation is fp32 in PSUM. V tiles are transposed by
the DMA xbar (16-bit transpose), not the PE.
"""
import math

import numpy as np
import ml_dtypes
from contextlib import ExitStack

import concourse.bass as bass
import concourse.mybir as mybir
import concourse.tile as tile
from concourse import bacc
from concourse.bass_utils import run_bass_kernel_spmd

F32 = mybir.dt.float32
F32R = mybir.dt.float32r
BF16 = mybir.dt.bfloat16
AF = mybir.ActivationFunctionType
ALU = mybir.AluOpType

P = 128
HID = 3072
HEADS = 24
HD = 128
MLP = 4 * HID            # 12288
L = 2048
NCORES = 8
H_PER = HEADS // NCORES  # 3 heads per core
DQK = H_PER * HD         # 384 q (and k, v) out dims per core
DMLP = MLP // NCORES     # 1536 mlp dims per core
DOUT1 = 3 * DQK + DMLP   # 2688 linear1 out dims per core
NBLK1 = DOUT1 // P       # 21 (0-2 q, 3-5 k, 6-8 v, 9-20 mlp)
NMLP = DMLP // P         # 12 mlp blocks
CATD = DQK + DMLP        # 1920 cat dims per core
NCAT = CATD // P         # 15
HC = HID // P            # 24 hidden chunks
NQ = 4                   # token quarters
LQ = L // NQ             # 512
LB = LQ // P             # 4 token tiles per quarter
NKB = L // P             # 16 key blocks
EPS = 1e-6

# linear1 block emission order: head-major (q0,k0,v0,...) so that attention
# head h's inputs complete as early as possible in the last quarter.
BLK_ORDER = [0, 3, 6, 1, 4, 7, 2, 5, 8] + list(range(9, NBLK1))

_CACHED = {}


def _build_nc():
    nc = bacc.Bacc("TRN2", target_bir_lowering=False, debug=False,
                   num_devices=NCORES)
    xT_in = nc.dram_tensor("xT_in", [HID, L], BF16, kind="ExternalInput")
    xsqT_in = nc.dram_tensor("xsqT_in", [HID, L], BF16, kind="ExternalInput")
    # cc2 rows 0-127 all cos (two stacked copies); ss2 all sin
    cc2_in = nc.dram_tensor("cc2_in", [P, L], BF16, kind="ExternalInput")
    ss2_in = nc.dram_tensor("ss2_in", [P, L], BF16, kind="ExternalInput")
    w1t_in = nc.dram_tensor("w1t_in", [NBLK1, P, HC, P], BF16,
                            kind="ExternalInput")
    b1_in = nc.dram_tensor("b1_in", [NBLK1, P], F32, kind="ExternalInput")
    w2t_in = nc.dram_tensor("w2t_in", [HC, P, NCAT, P], BF16,
                            kind="ExternalInput")
    b2_in = nc.dram_tensor("b2_in", [HC, P], F32, kind="ExternalInput")
    qs_in = nc.dram_tensor("qs_in", [HD], F32, kind="ExternalInput")
    ks_in = nc.dram_tensor("ks_in", [HD], F32, kind="ExternalInput")
    out_t = nc.dram_tensor("out_part", [HC, P, L], F32, kind="ExternalOutput")

    with tile.TileContext(nc) as tc, \
            nc.allow_low_precision(reason="bf16 matmul pipeline is intentional"):
        _emit(nc, tc, xT_in, xsqT_in, cc2_in, ss2_in, w1t_in, b1_in,
              w2t_in, b2_in, qs_in, ks_in, out_t)
    nc.compile()
    return nc


def _emit(nc, tc, xT_in, xsqT_in, cc2_in, ss2_in, w1t_in, b1_in,
          w2t_in, b2_in, qs_in, ks_in, out_t):
    with ExitStack() as top:
        const = top.enter_context(tc.tile_pool(name="const", bufs=1))
        psum = top.enter_context(tc.tile_pool(name="psum", bufs=4, space="PSUM"))
        pscol = top.enter_context(tc.tile_pool(name="pscol", bufs=4, space="PSUM"))

        # ---- constants ----
        ones_cb = const.tile([P, 1], BF16)       # K=128, M=1 column of ones
        nc.vector.memset(ones_cb, 1.0)
        ones_rb = const.tile([1, P], BF16)       # K=1, M=128 broadcast row
        nc.vector.memset(ones_rb, 1.0)
        sg = const.tile([P, 1], F32)             # rope sign: -1 top, +1 bottom
        nc.vector.memset(sg[0:64, :], -1.0)
        nc.vector.memset(sg[64:128, :], 1.0)
        eps_1 = const.tile([1, 1], F32)
        nc.vector.memset(eps_1, EPS)
        cc2 = const.tile([P, L], BF16)
        nc.sync.dma_start(out=cc2, in_=cc2_in[:, :])
        ss2 = const.tile([P, L], BF16)
        nc.sync.dma_start(out=ss2, in_=ss2_in[:, :])
        qs = const.tile([P, 1], F32)
        nc.sync.dma_start(out=qs, in_=qs_in[:, None])
        ks = const.tile([P, 1], F32)
        nc.sync.dma_start(out=ks, in_=ks_in[:, None])
        b1t = const.tile([P, NBLK1], F32)
        nc.sync.dma_start(out=b1t, in_=b1_in.rearrange("b p -> p b"))
        b2t = const.tile([P, HC], F32)
        nc.sync.dma_start(out=b2t, in_=b2_in.rearrange("b p -> p b"))

        glp = top.enter_context(tc.tile_pool(name="glp", bufs=1))
        gelT = [glp.tile([P, L], BF16, tag=f"gl{i}", name=f"gl{i}")
                for i in range(NMLP)]
        atp = top.enter_context(tc.tile_pool(name="attn", bufs=1))
        aoT = [atp.tile([P, L], BF16, tag=f"ao{h}", name=f"ao{h}")
               for h in range(H_PER)]

        with ExitStack() as bc_scope:
            qkv = bc_scope.enter_context(tc.tile_pool(name="qkv", bufs=1))
            qkT = [qkv.tile([P, L], BF16, tag=f"q{h}", name=f"q{h}")
                   for h in range(H_PER)] + \
                  [qkv.tile([P, L], BF16, tag=f"k{h}", name=f"k{h}")
                   for h in range(H_PER)]
            vblk = [[None] * NKB for _ in range(H_PER)]

            # ========================================================
            # Phase B: per quarter: LN stats -> normalize -> linear1
            # ========================================================
            with ExitStack() as bb:
                xrp = bb.enter_context(tc.tile_pool(name="xr", bufs=2))
                xsp = bb.enter_context(tc.tile_pool(name="xsq", bufs=6))
                w1p = bb.enter_context(tc.tile_pool(name="w1s", bufs=3))
                lnp = bb.enter_context(tc.tile_pool(name="ln", bufs=1))
                vqp = bb.enter_context(tc.tile_pool(name="vq", bufs=2))
                xT_r = xT_in.rearrange("(c p) l -> p c l", p=P)
                xsqT_r = xsqT_in.rearrange("(c p) l -> p c l", p=P)

                def prep_quarter(q):
                    """LN stats + in-place normalize of quarter q's x^T."""
                    qsl = slice(q * LQ, (q + 1) * LQ)
                    xq = xrp.tile([P, HC, LQ], BF16, tag="xq")
                    # per-chunk loads so the first column-sum matmuls can
                    # start as soon as the first chunk lands
                    for hc in range(HC):
                        nc.scalar.dma_start(out=xq[:, hc],
                                            in_=xT_r[:, hc, qsl])
                    psx = pscol.tile([1, LQ], F32, tag="col")
                    for hc in range(HC):
                        nc.tensor.matmul(psx, ones_cb, xq[:, hc],
                                         start=(hc == 0), stop=(hc == HC - 1))
                    psq = pscol.tile([1, LQ], F32, tag="col")
                    for hc in range(HC):
                        xsq = xsp.tile([P, LQ], BF16, tag="xsq")
                        nc.scalar.dma_start(out=xsq, in_=xsqT_r[:, hc, qsl])
                        nc.tensor.matmul(psq, ones_cb, xsq,
                                         start=(hc == 0), stop=(hc == HC - 1))
                    mu_row = lnp.tile([1, LQ], BF16, tag="mu")
                    nc.scalar.activation(mu_row, psx, AF.Copy, scale=1.0 / HID)
                    musq = lnp.tile([1, LQ], F32, tag="musq")
                    nc.scalar.square(musq, mu_row)
                    var = lnp.tile([1, LQ], F32, tag="var")
                    nc.vector.scalar_tensor_tensor(
                        var, psq, 1.0 / HID, musq, ALU.mult, ALU.subtract)
                    rinv = lnp.tile([1, LQ], BF16, tag="rinv")
                    nc.scalar.activation(rinv, var, AF.Abs_reciprocal_sqrt,
                                         bias=eps_1)
                    # broadcast mean and rstd down the partitions
                    pmu = psum.tile([P, LQ], F32, tag="big")
                    nc.tensor.matmul(pmu, ones_rb, mu_row, start=True, stop=True)
                    mu_b = lnp.tile([P, LQ], BF16, tag="mub")
                    nc.scalar.copy(mu_b, pmu)
                    prs = psum.tile([P, LQ], F32, tag="big")
                    nc.tensor.matmul(prs, ones_rb, rinv, start=True, stop=True)
                    r_b = lnp.tile([P, LQ], BF16, tag="rb")
                    nc.scalar.copy(r_b, prs)
                    # normalize in place: xq <- (xq - mu) * rstd
                    for hc in range(HC):
                        nc.vector.tensor_sub(xq[:, hc], xq[:, hc], mu_b)
                        nc.vector.tensor_mul(xq[:, hc], xq[:, hc], r_b)
                    return xq

                def lin1_block(q, xq, blk):
                    qsl = slice(q * LQ, (q + 1) * LQ)
                    w1t = w1p.tile([P, HC, P], BF16, tag="w1t")
                    nc.sync.dma_start(out=w1t, in_=w1t_in[blk])
                    ps = psum.tile([P, LQ], F32, tag="big")
                    for hc in range(HC):
                        nc.tensor.matmul(ps, w1t[:, hc], xq[:, hc],
                                         start=(hc == 0), stop=(hc == HC - 1))
                    if blk < 6:       # q / k -> qkT slices
                        nc.vector.tensor_scalar_add(
                            qkT[blk][:, qsl], ps, b1t[:, blk:blk + 1])
                    elif blk < 9:     # v: evict, then DMA-xbar transpose
                        h = blk - 6
                        vq = vqp.tile([P, LQ], BF16, tag="vq")
                        nc.vector.tensor_scalar_add(vq, ps,
                                                    b1t[:, blk:blk + 1])
                        for j in range(LB):
                            vb = qkv.tile([P, P], BF16,
                                          tag=f"vb{h}_{q * LB + j}",
                                          name=f"vb{h}_{q * LB + j}")
                            nc.sync.dma_start_transpose(
                                vb, vq[:, j * P:(j + 1) * P])
                            vblk[h][q * LB + j] = vb
                    else:             # mlp -> gelu, resident in SBUF
                        nc.scalar.activation(gelT[blk - 9][:, qsl], ps,
                                             AF.Gelu_apprx_tanh,
                                             bias=b1t[:, blk:blk + 1])

                # software-pipelined: quarter q+1's stats/normalize are
                # emitted a few blocks into quarter q's linear1, so the
                # x DMA and stats chain hide under the matmul stream.
                xq_prev = prep_quarter(0)
                for q in range(NQ):
                    for blk in BLK_ORDER[:3]:
                        lin1_block(q, xq_prev, blk)
                    if q + 1 < NQ:
                        xq_next = prep_quarter(q + 1)
                    for blk in BLK_ORDER[3:]:
                        lin1_block(q, xq_prev, blk)
                    if q + 1 < NQ:
                        xq_prev = xq_next

            # ========================================================
            # Phase C: QK-norm (RMS over head dim, on partitions) + rope,
            # emitted stage-major so per-tile chains pipeline across tiles.
            # ========================================================
            with ExitStack() as cc:
                rmsp = cc.enter_context(tc.tile_pool(name="rms", bufs=2))
                srp = cc.enter_context(tc.tile_pool(name="srp", bufs=8))
                rtp = cc.enter_context(tc.tile_pool(name="rtp", bufs=4))
                ptp = cc.enter_context(tc.tile_pool(name="ptp", bufs=34))
                sdp = cc.enter_context(tc.tile_pool(name="sdp", bufs=2))

                # C tiles in the order attention consumes them: q0,k0,q1,...
                CT = []
                for h in range(H_PER):
                    CT.append((qkT[h], qs))
                    CT.append((qkT[H_PER + h], ks))
                cst = [{} for _ in range(6)]

                def c_sq(i):
                    # on GpSimd/Pool: its queue is empty, so the square runs
                    # the moment qkT[i] is complete, mid-linear1 — the DVE
                    # queue is still draining linear1 evictions then.
                    t, _ = CT[i]
                    sq = rmsp.tile([P, L], BF16, tag="sq")
                    nc.gpsimd.tensor_mul(sq, t, t)
                    cst[i]["sq"] = sq

                def c_sums(i):
                    rinvs = []
                    for j in range(NQ):
                        jsl = slice(j * LQ, (j + 1) * LQ)
                        pc = pscol.tile([1, LQ], F32, tag="col")
                        nc.tensor.matmul(pc, ones_cb, cst[i]["sq"][:, jsl],
                                         start=True, stop=True)
                        rinv = srp.tile([1, LQ], BF16, tag="rinv")
                        nc.scalar.activation(rinv, pc, AF.Abs_reciprocal_sqrt,
                                             bias=eps_1, scale=1.0 / HD)
                        rinvs.append(rinv)
                    cst[i]["rinvs"] = rinvs

                def c_bcast(i):
                    rb = rmsp.tile([P, L], BF16, tag="rb")
                    for j, rinv in enumerate(cst[i]["rinvs"]):
                        jsl = slice(j * LQ, (j + 1) * LQ)
                        pb = psum.tile([P, LQ], F32, tag="big")
                        nc.tensor.matmul(pb, ones_rb, rinv,
                                         start=True, stop=True)
                        nc.scalar.copy(rb[:, jsl], pb)
                    cst[i]["rb"] = rb

                def c_finish(i):
                    t, scale_ap = CT[i]
                    # t <- (t * head_scale) * rinv_broadcast in one DVE pass
                    nc.vector.scalar_tensor_tensor(
                        t, t, scale_ap, cst[i]["rb"], ALU.mult, ALU.mult)
                    # rope: rows 0-63 even pair components, 64-127 odd
                    A = rtp.tile([P, L], BF16, tag="rt")   # t * cos
                    B = rtp.tile([P, L], BF16, tag="rt")   # t * sin
                    Bx = rtp.tile([P, L], BF16, tag="rt")  # halves swapped
                    nc.vector.tensor_mul(A, t, cc2)
                    nc.vector.tensor_mul(B, t, ss2)
                    nc.scalar.dma_start(out=Bx[0:64, :], in_=B[64:128, :])
                    nc.scalar.dma_start(out=Bx[64:128, :], in_=B[0:64, :])
                    # t <- A + sg * Bx  (top: -swap, bottom: +swap)
                    nc.vector.scalar_tensor_tensor(
                        t, Bx, sg, A, ALU.mult, ALU.add)

                for i in range(6):
                    c_sq(i)
                c_sums(0), c_sums(1)
                c_bcast(0), c_bcast(1)
                c_finish(0), c_finish(1)
                # tiles 2..5 are finished inside the attention loop below,
                # hiding their chains under head h-1's matmuls

                # ====================================================
                # Phase E: attention per head, one-quarter lookahead so
                # the Exp (scalar engine) hides under PE work
                # ====================================================
                def scores(h, qc):
                    qT, kT = qkT[h], qkT[H_PER + h]
                    qslc = slice(qc * LQ, (qc + 1) * LQ)
                    pts = []
                    for kb in range(NKB):
                        ps = psum.tile([P, LQ], F32, tag="big")
                        nc.tensor.matmul(ps, kT[:, kb * P:(kb + 1) * P],
                                         qT[:, qslc], start=True, stop=True)
                        ptile = ptp.tile([P, LQ], BF16, tag="pt", name="pt")
                        nc.scalar.activation(ptile, ps, AF.Exp)
                        pts.append(ptile)
                    return pts

                def finish(h, qc, pts):
                    qslc = slice(qc * LQ, (qc + 1) * LQ)
                    psd = pscol.tile([1, LQ], F32, tag="col")
                    for kb in range(NKB):
                        nc.tensor.matmul(psd, ones_cb, pts[kb],
                                         start=(kb == 0), stop=(kb == NKB - 1))
                    rdf = sdp.tile([1, LQ], F32, tag="rdf")
                    nc.vector.reciprocal_approx_fast(rdf, psd)
                    rd = sdp.tile([1, LQ], BF16, tag="rd")
                    nc.scalar.copy(rd, rdf)
                    # attn @ V accumulation first: it does not depend on the
                    # reciprocal chain, so the PE never waits for it
                    pso = psum.tile([P, LQ], F32, tag="big")
                    for kb in range(NKB):
                        nc.tensor.matmul(pso, vblk[h][kb], pts[kb],
                                         start=(kb == 0), stop=(kb == NKB - 1))
                    pbd = psum.tile([P, LQ], F32, tag="big")
                    nc.tensor.matmul(pbd, ones_rb, rd, start=True, stop=True)
                    rbd = sdp.tile([P, LQ], BF16, tag="rbd")
                    nc.scalar.copy(rbd, pbd)
                    nc.vector.tensor_mul(aoT[h][:, qslc], pso, rbd)

                prev = None
                for h in range(H_PER):
                    for qc in range(NQ):
                        pts = scores(h, qc)
                        if qc == 0 and h + 1 < H_PER:
                            # prepare head h+1's q/k while head h runs
                            c_sums(2 * h + 2), c_sums(2 * h + 3)
                        elif qc == 1 and h + 1 < H_PER:
                            c_bcast(2 * h + 2), c_bcast(2 * h + 3)
                        elif qc == 2 and h + 1 < H_PER:
                            c_finish(2 * h + 2), c_finish(2 * h + 3)
                        if prev is not None:
                            finish(*prev)
                        prev = (h, qc, pts)
                finish(*prev)

        # ========================================================
        # Phase F: linear2 (row-parallel partial); +b2 on core 0
        # ========================================================
        with ExitStack() as fc:
            w2p = fc.enter_context(tc.tile_pool(name="w2p", bufs=3))
            otp = fc.enter_context(tc.tile_pool(name="otp", bufs=4))
            catT = aoT + gelT  # 15 chunks of [128, L]
            for blk in range(HC):
                w2t = w2p.tile([P, NCAT, P], BF16, tag="w2t")
                nc.sync.dma_start(out=w2t, in_=w2t_in[blk])
                for lc in range(NQ):
                    lsl = slice(lc * LQ, (lc + 1) * LQ)
                    ps = psum.tile([P, LQ], F32, tag="big")
                    for cc2_ in range(NCAT):
                        nc.tensor.matmul(ps, w2t[:, cc2_], catT[cc2_][:, lsl],
                                         start=(cc2_ == 0),
                                         stop=(cc2_ == NCAT - 1))
                    ot = otp.tile([P, LQ], F32, tag="ot")
                    nc.vector.tensor_scalar_add(ot, ps, b2t[:, blk:blk + 1])
                    nc.sync.dma_start(out=out_t[blk, :, lsl], in_=ot)


def _bf16(a):
    return np.ascontiguousarray(a.astype(ml_dtypes.bfloat16))


def _host_prep(inputs):
    f = np.float32
    perm = np.concatenate([np.arange(0, HD, 2), np.arange(1, HD, 2)])
    # ---- LoRA folds ----
    w1_eff = inputs["w1"].astype(f).copy()
    for i, nm in enumerate(("q", "k", "v")):
        up = inputs[f"lora_{nm}_up"].astype(f)
        dn = inputs[f"lora_{nm}_down"].astype(f)
        w1_eff[i * HID:(i + 1) * HID] += up @ dn
    w2_eff = inputs["w2"].astype(f) + \
        inputs["proj_up"].astype(f) @ inputs["proj_down"].astype(f)
    # ---- modulation fold (shift/scale/gate from vec) ----
    vec = inputs["vec"].astype(np.float64).reshape(HID)
    sv = (vec / (1.0 + np.exp(-vec)))
    m = sv @ inputs["mod_w"].astype(np.float64).T + \
        inputs["mod_b"].astype(np.float64)
    shift, scale, gate = np.split(m.astype(f), 3)
    b1_eff = inputs["b1"].astype(f) + w1_eff @ shift
    w1s = w1_eff * (1.0 + scale)[None, :]
    w2g = w2_eff * gate[:, None]
    b2g = inputs["b2"].astype(f) * gate

    wq = w1s[0:HID].reshape(HEADS, HD, HID)[:, perm, :]
    wk = w1s[HID:2 * HID].reshape(HEADS, HD, HID)[:, perm, :]
    wv = w1s[2 * HID:3 * HID].reshape(HEADS, HD, HID)
    wm = w1s[3 * HID:].reshape(NCORES, DMLP, HID)
    bq = b1_eff[0:HID].reshape(HEADS, HD)[:, perm]
    bk = b1_eff[HID:2 * HID].reshape(HEADS, HD)[:, perm]
    bv = b1_eff[2 * HID:3 * HID].reshape(HEADS, HD)
    bm = b1_eff[3 * HID:].reshape(NCORES, DMLP)

    pe = inputs["pe"].astype(f)
    cos = pe[0, 0, :, :, 0, 0]   # (L, 64)
    sin = pe[0, 0, :, :, 1, 0]   # (L, 64)
    cc2 = _bf16(np.concatenate([cos.T, cos.T], axis=0))   # (128, L)
    ss2 = _bf16(np.concatenate([sin.T, sin.T], axis=0))   # (128, L)

    qsc = inputs["q_scale"].astype(f)[perm] / math.sqrt(HD)
    ksc = inputs["k_scale"].astype(f)[perm]
    x2d = inputs["x"].astype(f).reshape(L, HID)
    xb = x2d.astype(ml_dtypes.bfloat16)
    xT = np.ascontiguousarray(xb.T)
    xsqT = np.ascontiguousarray(
        (xb.astype(f) ** 2).astype(ml_dtypes.bfloat16).T)

    in_maps = []
    for c in range(NCORES):
        hs = slice(H_PER * c, H_PER * (c + 1))
        w1c = np.concatenate([
            wq[hs].reshape(DQK, HID), wk[hs].reshape(DQK, HID),
            wv[hs].reshape(DQK, HID), wm[c]], axis=0)      # (2688, 3072)
        b1c = np.concatenate([
            bq[hs].reshape(DQK), bk[hs].reshape(DQK), bv[hs].reshape(DQK),
            bm[c]])
        w2c = np.concatenate([
            w2g[:, DQK * c:DQK * (c + 1)],
            w2g[:, HID + DMLP * c:HID + DMLP * (c + 1)]], axis=1)  # (3072,1920)
        b2c = b2g if c == 0 else np.zeros_like(b2g)
        w1tile = _bf16(w1c.reshape(NBLK1, P, HC, P).transpose(0, 3, 2, 1))
        w2tile = _bf16(w2c.reshape(HC, P, NCAT, P).transpose(0, 3, 2, 1))
        in_maps.append({
            "xT_in": xT,
            "xsqT_in": xsqT,
            "cc2_in": cc2,
            "ss2_in": ss2,
            "w1t_in": w1tile,
            "b1_in": np.ascontiguousarray(b1c.reshape(NBLK1, P)),
            "w2t_in": w2tile,
            "b2_in": np.ascontiguousarray(b2c.reshape(HC, P)),
            "qs_in": qsc,
            "ks_in": ksc,
        })
    return in_maps


def kernel(**inputs):
    if "nc" not in _CACHED:
        _CACHED["nc"] = _build_nc()
    nc = _CACHED["nc"]
    in_maps = _host_prep(inputs)
    res = run_bass_kernel_spmd(nc, in_maps, core_ids=list(range(NCORES)))
    acc = np.zeros((HID, L), dtype=np.float64)
    for c in range(NCORES):
        acc += res.results[c]["out_part"].reshape(HID, L)
    out = inputs["x"].astype(np.float32).reshape(L, HID) + acc.T.astype(np.float32)
    return out.reshape(1, L, HID).astype(np.float32)


# revision 32
# speedup vs baseline: 1.0567x; 1.0567x over previous
"""DreamFit single-stream processor block on 8 Trainium2 NeuronCores.

Sharding: 3 heads per core for SDPA (column-parallel linear1), 1/8 of the MLP
per core, row-parallel linear2 (host sums the 8 partial outputs).

Host-side folds (all cheap numpy): LoRA branches into w1/w2 (lora_weight==1),
the modulation (shift/scale/gate from vec) into w1 columns / b1 / w2 rows, so
the device kernel is just: LN-normalize -> linear1 -> qknorm/rope -> SDPA ->
gelu -> linear2 partial. x is passed pre-transposed (and pre-squared, for the
LN stats) so no on-device transposes of x are needed; LN stats come from PE
column-sum matmuls. All matmul operands are bf16 (full PE rate, fast weight
load, half the DMA); accumulation is fp32 in PSUM. V tiles are transposed by
the DMA xbar (16-bit transpose), not the PE.
"""
import math

import numpy as np
import ml_dtypes
from contextlib import ExitStack

import concourse.bass as bass
import concourse.mybir as mybir
import concourse.tile as tile
from concourse import bacc
from concourse.bass_utils import run_bass_kernel_spmd

F32 = mybir.dt.float32
F32R = mybir.dt.float32r
BF16 = mybir.dt.bfloat16
AF = mybir.ActivationFunctionType
ALU = mybir.AluOpType

P = 128
HID = 3072
HEADS = 24
HD = 128
MLP = 4 * HID            # 12288
L = 2048
NCORES = 8
H_PER = HEADS // NCORES  # 3 heads per core
DQK = H_PER * HD         # 384 q (and k, v) out dims per core
DMLP = MLP // NCORES     # 1536 mlp dims per core
DOUT1 = 3 * DQK + DMLP   # 2688 linear1 out dims per core
NBLK1 = DOUT1 // P       # 21 (0-2 q, 3-5 k, 6-8 v, 9-20 mlp)
NMLP = DMLP // P         # 12 mlp blocks
CATD = DQK + DMLP        # 1920 cat dims per core
NCAT = CATD // P         # 15
HC = HID // P            # 24 hidden chunks
NQ = 4                   # token quarters
LQ = L // NQ             # 512
LB = LQ // P             # 4 token tiles per quarter
NKB = L // P             # 16 key blocks
EPS = 1e-6

# linear1 block emission order: head-major (q0,k0,v0,...) so that attention
# head h's inputs complete as early as possible in the last quarter.
BLK_ORDER = [0, 3, 6, 1, 4, 7, 2, 5, 8] + list(range(9, NBLK1))

_CACHED = {}


def _build_nc():
    nc = bacc.Bacc("TRN2", target_bir_lowering=False, debug=False,
                   num_devices=NCORES)
    xT_in = nc.dram_tensor("xT_in", [HID, L], BF16, kind="ExternalInput")
    xsqT_in = nc.dram_tensor("xsqT_in", [HID, L], BF16, kind="ExternalInput")
    # cc2 rows 0-127 all cos (two stacked copies); ss2 all sin
    cc2_in = nc.dram_tensor("cc2_in", [P, L], BF16, kind="ExternalInput")
    ss2_in = nc.dram_tensor("ss2_in", [P, L], BF16, kind="ExternalInput")
    w1t_in = nc.dram_tensor("w1t_in", [NBLK1, P, HC, P], BF16,
                            kind="ExternalInput")
    b1_in = nc.dram_tensor("b1_in", [NBLK1, P], F32, kind="ExternalInput")
    w2t_in = nc.dram_tensor("w2t_in", [HC, P, NCAT, P], BF16,
                            kind="ExternalInput")
    b2_in = nc.dram_tensor("b2_in", [HC, P], F32, kind="ExternalInput")
    qs_in = nc.dram_tensor("qs_in", [HD], F32, kind="ExternalInput")
    ks_in = nc.dram_tensor("ks_in", [HD], F32, kind="ExternalInput")
    out_t = nc.dram_tensor("out_part", [HC, P, L], F32, kind="ExternalOutput")

    with tile.TileContext(nc) as tc, \
            nc.allow_low_precision(reason="bf16 matmul pipeline is intentional"):
        _emit(nc, tc, xT_in, xsqT_in, cc2_in, ss2_in, w1t_in, b1_in,
              w2t_in, b2_in, qs_in, ks_in, out_t)
    nc.compile()
    return nc


def _emit(nc, tc, xT_in, xsqT_in, cc2_in, ss2_in, w1t_in, b1_in,
          w2t_in, b2_in, qs_in, ks_in, out_t):
    with ExitStack() as top:
        const = top.enter_context(tc.tile_pool(name="const", bufs=1))
        psum = top.enter_context(tc.tile_pool(name="psum", bufs=4, space="PSUM"))
        pscol = top.enter_context(tc.tile_pool(name="pscol", bufs=4, space="PSUM"))

        # ---- constants ----
        ones_cb = const.tile([P, 1], BF16)       # K=128, M=1 column of ones
        nc.vector.memset(ones_cb, 1.0)
        ones_rb = const.tile([1, P], BF16)       # K=1, M=128 broadcast row
        nc.vector.memset(ones_rb, 1.0)
        sg = const.tile([P, 1], F32)             # rope sign: -1 top, +1 bottom
        nc.vector.memset(sg[0:64, :], -1.0)
        nc.vector.memset(sg[64:128, :], 1.0)
        eps_1 = const.tile([1, 1], F32)
        nc.vector.memset(eps_1, EPS)
        cc2 = const.tile([P, L], BF16)
        nc.sync.dma_start(out=cc2, in_=cc2_in[:, :])
        ss2 = const.tile([P, L], BF16)
        nc.sync.dma_start(out=ss2, in_=ss2_in[:, :])
        qs = const.tile([P, 1], F32)
        nc.sync.dma_start(out=qs, in_=qs_in[:, None])
        ks = const.tile([P, 1], F32)
        nc.sync.dma_start(out=ks, in_=ks_in[:, None])
        b1t = const.tile([P, NBLK1], F32)
        nc.sync.dma_start(out=b1t, in_=b1_in.rearrange("b p -> p b"))
        b2t = const.tile([P, HC], F32)
        nc.sync.dma_start(out=b2t, in_=b2_in.rearrange("b p -> p b"))

        glp = top.enter_context(tc.tile_pool(name="glp", bufs=1))
        gelT = [glp.tile([P, L], BF16, tag=f"gl{i}", name=f"gl{i}")
                for i in range(NMLP)]
        atp = top.enter_context(tc.tile_pool(name="attn", bufs=1))
        aoT = [atp.tile([P, L], BF16, tag=f"ao{h}", name=f"ao{h}")
               for h in range(H_PER)]

        with ExitStack() as bc_scope:
            qkv = bc_scope.enter_context(tc.tile_pool(name="qkv", bufs=1))
            qkT = [qkv.tile([P, L], BF16, tag=f"q{h}", name=f"q{h}")
                   for h in range(H_PER)] + \
                  [qkv.tile([P, L], BF16, tag=f"k{h}", name=f"k{h}")
                   for h in range(H_PER)]
            vblk = [[None] * NKB for _ in range(H_PER)]

            # ========================================================
            # Phase B: per quarter: LN stats -> normalize -> linear1
            # ========================================================
            with ExitStack() as bb:
                xrp = bb.enter_context(tc.tile_pool(name="xr", bufs=2))
                xsp = bb.enter_context(tc.tile_pool(name="xsq", bufs=1))
                w1p = bb.enter_context(tc.tile_pool(name="w1s", bufs=3))
                lnp = bb.enter_context(tc.tile_pool(name="ln", bufs=1))
                vqp = bb.enter_context(tc.tile_pool(name="vq", bufs=2))
                xT_r = xT_in.rearrange("(c p) l -> p c l", p=P)
                xsqT_r = xsqT_in.rearrange("(c p) l -> p c l", p=P)

                def prep_stats(q):
                    """Load quarter q's x^T and compute LN row stats."""
                    qsl = slice(q * LQ, (q + 1) * LQ)
                    xq = xrp.tile([P, HC, LQ], BF16, tag="xq")
                    if q == 0:
                        # per-chunk loads so the first column-sum matmuls
                        # start as soon as the first chunk lands
                        for hc in range(HC):
                            nc.scalar.dma_start(out=xq[:, hc],
                                                in_=xT_r[:, hc, qsl])
                    else:
                        nc.scalar.dma_start(out=xq, in_=xT_r[:, :, qsl])
                    psx = pscol.tile([1, LQ], F32, tag="col")
                    for hc in range(HC):
                        nc.tensor.matmul(psx, ones_cb, xq[:, hc],
                                         start=(hc == 0), stop=(hc == HC - 1))
                    psq = pscol.tile([1, LQ], F32, tag="col")
                    xsq = xsp.tile([P, HC, LQ], BF16, tag="xsqb")
                    if q == 0:
                        for hc in range(HC):
                            nc.scalar.dma_start(out=xsq[:, hc],
                                                in_=xsqT_r[:, hc, qsl])
                    else:
                        nc.scalar.dma_start(out=xsq, in_=xsqT_r[:, :, qsl])
                    for hc in range(HC):
                        nc.tensor.matmul(psq, ones_cb, xsq[:, hc],
                                         start=(hc == 0), stop=(hc == HC - 1))
                    mu_row = lnp.tile([1, LQ], BF16, tag="mu")
                    nc.scalar.activation(mu_row, psx, AF.Copy, scale=1.0 / HID)
                    musq = lnp.tile([1, LQ], F32, tag="musq")
                    nc.scalar.square(musq, mu_row)
                    var = lnp.tile([1, LQ], F32, tag="var")
                    nc.vector.scalar_tensor_tensor(
                        var, psq, 1.0 / HID, musq, ALU.mult, ALU.subtract)
                    rinv = lnp.tile([1, LQ], BF16, tag="rinv")
                    nc.scalar.activation(rinv, var, AF.Abs_reciprocal_sqrt,
                                         bias=eps_1)
                    return xq, mu_row, rinv

                def prep_norm(state):
                    """Broadcast the stats and normalize x^T in place."""
                    xq, mu_row, rinv = state
                    pmu = psum.tile([P, LQ], F32, tag="big")
                    nc.tensor.matmul(pmu, ones_rb, mu_row, start=True, stop=True)
                    mu_b = lnp.tile([P, LQ], BF16, tag="mub")
                    nc.scalar.copy(mu_b, pmu)
                    prs = psum.tile([P, LQ], F32, tag="big")
                    nc.tensor.matmul(prs, ones_rb, rinv, start=True, stop=True)
                    r_b = lnp.tile([P, LQ], BF16, tag="rb")
                    nc.scalar.copy(r_b, prs)
                    # normalize in place: xq <- (xq - mu) * rstd
                    for hc in range(HC):
                        nc.vector.tensor_sub(xq[:, hc], xq[:, hc], mu_b)
                        nc.vector.tensor_mul(xq[:, hc], xq[:, hc], r_b)
                    return xq

                def lin1_block(q, xq, blk):
                    qsl = slice(q * LQ, (q + 1) * LQ)
                    w1t = w1p.tile([P, HC, P], BF16, tag="w1t")
                    nc.sync.dma_start(out=w1t, in_=w1t_in[blk])
                    ps = psum.tile([P, LQ], F32, tag="big")
                    for hc in range(HC):
                        nc.tensor.matmul(ps, w1t[:, hc], xq[:, hc],
                                         start=(hc == 0), stop=(hc == HC - 1))
                    if blk < 6:       # q / k -> qkT slices
                        nc.vector.tensor_scalar_add(
                            qkT[blk][:, qsl], ps, b1t[:, blk:blk + 1])
                    elif blk < 9:     # v: evict, then DMA-xbar transpose
                        h = blk - 6
                        vq = vqp.tile([P, LQ], BF16, tag="vq")
                        nc.vector.tensor_scalar_add(vq, ps,
                                                    b1t[:, blk:blk + 1])
                        for j in range(LB):
                            vb = qkv.tile([P, P], BF16,
                                          tag=f"vb{h}_{q * LB + j}",
                                          name=f"vb{h}_{q * LB + j}")
                            nc.scalar.dma_start_transpose(
                                vb, vq[:, j * P:(j + 1) * P])
                            vblk[h][q * LB + j] = vb
                    else:             # mlp -> gelu, resident in SBUF
                        nc.scalar.activation(gelT[blk - 9][:, qsl], ps,
                                             AF.Gelu_apprx_tanh,
                                             bias=b1t[:, blk:blk + 1])

                # software-pipelined: quarter q+1's stats and normalize are
                # emitted a few blocks into quarter q's linear1, so the
                # x DMA and stats chain hide under the matmul stream.
                xq_prev = prep_norm(prep_stats(0))
                for q in range(NQ):
                    for blk in BLK_ORDER[:3]:
                        lin1_block(q, xq_prev, blk)
                    if q + 1 < NQ:
                        st_next = prep_stats(q + 1)
                    for blk in BLK_ORDER[3:6]:
                        lin1_block(q, xq_prev, blk)
                    if q + 1 < NQ:
                        xq_next = prep_norm(st_next)
                    for blk in BLK_ORDER[6:]:
                        lin1_block(q, xq_prev, blk)
                    if q + 1 < NQ:
                        xq_prev = xq_next

            # ========================================================
            # Phase C: QK-norm (RMS over head dim, on partitions) + rope,
            # emitted stage-major so per-tile chains pipeline across tiles.
            # ========================================================
            with ExitStack() as cc:
                rmsp = cc.enter_context(tc.tile_pool(name="rms", bufs=2))
                srp = cc.enter_context(tc.tile_pool(name="srp", bufs=8))
                rtp = cc.enter_context(tc.tile_pool(name="rtp", bufs=4))
                ptp = cc.enter_context(tc.tile_pool(name="ptp", bufs=34))
                sdp = cc.enter_context(tc.tile_pool(name="sdp", bufs=2))

                # C tiles in the order attention consumes them: q0,k0,q1,...
                CT = []
                for h in range(H_PER):
                    CT.append((qkT[h], qs))
                    CT.append((qkT[H_PER + h], ks))
                cst = [{} for _ in range(6)]

                def c_sq(i):
                    t, _ = CT[i]
                    sq = rmsp.tile([P, L], BF16, tag="sq")
                    nc.vector.tensor_mul(sq, t, t)
                    cst[i]["sq"] = sq

                def c_sums(i):
                    rinvs = []
                    for j in range(NQ):
                        jsl = slice(j * LQ, (j + 1) * LQ)
                        pc = pscol.tile([1, LQ], F32, tag="col")
                        nc.tensor.matmul(pc, ones_cb, cst[i]["sq"][:, jsl],
                                         start=True, stop=True)
                        rinv = srp.tile([1, LQ], BF16, tag="rinv")
                        nc.scalar.activation(rinv, pc, AF.Abs_reciprocal_sqrt,
                                             bias=eps_1, scale=1.0 / HD)
                        rinvs.append(rinv)
                    cst[i]["rinvs"] = rinvs

                def c_bcast(i):
                    rb = rmsp.tile([P, L], BF16, tag="rb")
                    for j, rinv in enumerate(cst[i]["rinvs"]):
                        jsl = slice(j * LQ, (j + 1) * LQ)
                        pb = psum.tile([P, LQ], F32, tag="big")
                        nc.tensor.matmul(pb, ones_rb, rinv,
                                         start=True, stop=True)
                        nc.scalar.copy(rb[:, jsl], pb)
                    cst[i]["rb"] = rb

                def c_finish(i):
                    t, scale_ap = CT[i]
                    # t <- (t * head_scale) * rinv_broadcast in one DVE pass
                    nc.vector.scalar_tensor_tensor(
                        t, t, scale_ap, cst[i]["rb"], ALU.mult, ALU.mult)
                    # rope: rows 0-63 even pair components, 64-127 odd
                    A = rtp.tile([P, L], BF16, tag="rt")   # t * cos
                    B = rtp.tile([P, L], BF16, tag="rt")   # t * sin
                    Bx = rtp.tile([P, L], BF16, tag="rt")  # halves swapped
                    nc.vector.tensor_mul(A, t, cc2)
                    nc.vector.tensor_mul(B, t, ss2)
                    nc.scalar.dma_start(out=Bx[0:64, :], in_=B[64:128, :])
                    nc.scalar.dma_start(out=Bx[64:128, :], in_=B[0:64, :])
                    # t <- A + sg * Bx  (top: -swap, bottom: +swap)
                    nc.vector.scalar_tensor_tensor(
                        t, Bx, sg, A, ALU.mult, ALU.add)

                for i in range(6):
                    c_sq(i)
                c_sums(0), c_sums(1)
                c_bcast(0), c_bcast(1)
                c_finish(0), c_finish(1)
                # tiles 2..5 are finished inside the attention loop below,
                # hiding their chains under head h-1's matmuls

                # ====================================================
                # Phase E: attention per head, one-quarter lookahead so
                # the Exp (scalar engine) hides under PE work
                # ====================================================
                def scores(h, qc):
                    qT, kT = qkT[h], qkT[H_PER + h]
                    qslc = slice(qc * LQ, (qc + 1) * LQ)
                    pts = []
                    for kb in range(NKB):
                        ps = psum.tile([P, LQ], F32, tag="big")
                        nc.tensor.matmul(ps, kT[:, kb * P:(kb + 1) * P],
                                         qT[:, qslc], start=True, stop=True)
                        ptile = ptp.tile([P, LQ], BF16, tag="pt", name="pt")
                        nc.scalar.activation(ptile, ps, AF.Exp)
                        pts.append(ptile)
                    return pts

                def finish(h, qc, pts):
                    qslc = slice(qc * LQ, (qc + 1) * LQ)
                    psd = pscol.tile([1, LQ], F32, tag="col")
                    for kb in range(NKB):
                        nc.tensor.matmul(psd, ones_cb, pts[kb],
                                         start=(kb == 0), stop=(kb == NKB - 1))
                    rdf = sdp.tile([1, LQ], F32, tag="rdf")
                    nc.vector.reciprocal_approx_fast(rdf, psd)
                    rd = sdp.tile([1, LQ], BF16, tag="rd")
                    nc.scalar.copy(rd, rdf)
                    # attn @ V accumulation first: it does not depend on the
                    # reciprocal chain, so the PE never waits for it
                    pso = psum.tile([P, LQ], F32, tag="big")
                    for kb in range(NKB):
                        nc.tensor.matmul(pso, vblk[h][kb], pts[kb],
                                         start=(kb == 0), stop=(kb == NKB - 1))
                    pbd = psum.tile([P, LQ], F32, tag="big")
                    nc.tensor.matmul(pbd, ones_rb, rd, start=True, stop=True)
                    rbd = sdp.tile([P, LQ], BF16, tag="rbd")
                    nc.scalar.copy(rbd, pbd)
                    nc.vector.tensor_mul(aoT[h][:, qslc], pso, rbd)

                prev = None
                for h in range(H_PER):
                    for qc in range(NQ):
                        pts = scores(h, qc)
                        if qc == 0 and h + 1 < H_PER:
                            # prepare head h+1's q/k while head h runs
                            c_sums(2 * h + 2), c_sums(2 * h + 3)
                        elif qc == 1 and h + 1 < H_PER:
                            c_bcast(2 * h + 2), c_bcast(2 * h + 3)
                        elif qc == 2 and h + 1 < H_PER:
                            c_finish(2 * h + 2), c_finish(2 * h + 3)
                        if prev is not None:
                            finish(*prev)
                        prev = (h, qc, pts)
                finish(*prev)

        # ========================================================
        # Phase F: linear2 (row-parallel partial); +b2 on core 0
        # ========================================================
        with ExitStack() as fc:
            w2p = fc.enter_context(tc.tile_pool(name="w2p", bufs=3))
            otp = fc.enter_context(tc.tile_pool(name="otp", bufs=4))
            catT = aoT + gelT  # 15 chunks of [128, L]
            for blk in range(HC):
                w2t = w2p.tile([P, NCAT, P], BF16, tag="w2t")
                nc.sync.dma_start(out=w2t, in_=w2t_in[blk])
                for lc in range(NQ):
                    lsl = slice(lc * LQ, (lc + 1) * LQ)
                    ps = psum.tile([P, LQ], F32, tag="big")
                    for cc2_ in range(NCAT):
                        nc.tensor.matmul(ps, w2t[:, cc2_], catT[cc2_][:, lsl],
                                         start=(cc2_ == 0),
                                         stop=(cc2_ == NCAT - 1))
                    ot = otp.tile([P, LQ], F32, tag="ot")
                    nc.vector.tensor_scalar_add(ot, ps, b2t[:, blk:blk + 1])
                    nc.sync.dma_start(out=out_t[blk, :, lsl], in_=ot)


def _bf16(a):
    return np.ascontiguousarray(a.astype(ml_dtypes.bfloat16))


def _host_prep(inputs):
    f = np.float32
    perm = np.concatenate([np.arange(0, HD, 2), np.arange(1, HD, 2)])
    # ---- LoRA folds ----
    w1_eff = inputs["w1"].astype(f).copy()
    for i, nm in enumerate(("q", "k", "v")):
        up = inputs[f"lora_{nm}_up"].astype(f)
        dn = inputs[f"lora_{nm}_down"].astype(f)
        w1_eff[i * HID:(i + 1) * HID] += up @ dn
    w2_eff = inputs["w2"].astype(f) + \
        inputs["proj_up"].astype(f) @ inputs["proj_down"].astype(f)
    # ---- modulation fold (shift/scale/gate from vec) ----
    vec = inputs["vec"].astype(np.float64).reshape(HID)
    sv = (vec / (1.0 + np.exp(-vec)))
    m = sv @ inputs["mod_w"].astype(np.float64).T + \
        inputs["mod_b"].astype(np.float64)
    shift, scale, gate = np.split(m.astype(f), 3)
    b1_eff = inputs["b1"].astype(f) + w1_eff @ shift
    w1s = w1_eff * (1.0 + scale)[None, :]
    w2g = w2_eff * gate[:, None]
    b2g = inputs["b2"].astype(f) * gate

    wq = w1s[0:HID].reshape(HEADS, HD, HID)[:, perm, :]
    wk = w1s[HID:2 * HID].reshape(HEADS, HD, HID)[:, perm, :]
    wv = w1s[2 * HID:3 * HID].reshape(HEADS, HD, HID)
    wm = w1s[3 * HID:].reshape(NCORES, DMLP, HID)
    bq = b1_eff[0:HID].reshape(HEADS, HD)[:, perm]
    bk = b1_eff[HID:2 * HID].reshape(HEADS, HD)[:, perm]
    bv = b1_eff[2 * HID:3 * HID].reshape(HEADS, HD)
    bm = b1_eff[3 * HID:].reshape(NCORES, DMLP)

    pe = inputs["pe"].astype(f)
    cos = pe[0, 0, :, :, 0, 0]   # (L, 64)
    sin = pe[0, 0, :, :, 1, 0]   # (L, 64)
    cc2 = _bf16(np.concatenate([cos.T, cos.T], axis=0))   # (128, L)
    ss2 = _bf16(np.concatenate([sin.T, sin.T], axis=0))   # (128, L)

    qsc = inputs["q_scale"].astype(f)[perm] / math.sqrt(HD)
    ksc = inputs["k_scale"].astype(f)[perm]
    x2d = inputs["x"].astype(f).reshape(L, HID)
    xb = x2d.astype(ml_dtypes.bfloat16)
    xT = np.ascontiguousarray(xb.T)
    xsqT = np.ascontiguousarray(
        (xb.astype(f) ** 2).astype(ml_dtypes.bfloat16).T)

    in_maps = []
    for c in range(NCORES):
        hs = slice(H_PER * c, H_PER * (c + 1))
        w1c = np.concatenate([
            wq[hs].reshape(DQK, HID), wk[hs].reshape(DQK, HID),
            wv[hs].reshape(DQK, HID), wm[c]], axis=0)      # (2688, 3072)
        b1c = np.concatenate([
            bq[hs].reshape(DQK), bk[hs].reshape(DQK), bv[hs].reshape(DQK),
            bm[c]])
        w2c = np.concatenate([
            w2g[:, DQK * c:DQK * (c + 1)],
            w2g[:, HID + DMLP * c:HID + DMLP * (c + 1)]], axis=1)  # (3072,1920)
        b2c = b2g if c == 0 else np.zeros_like(b2g)
        w1tile = _bf16(w1c.reshape(NBLK1, P, HC, P).transpose(0, 3, 2, 1))
        w2tile = _bf16(w2c.reshape(HC, P, NCAT, P).transpose(0, 3, 2, 1))
        in_maps.append({
            "xT_in": xT,
            "xsqT_in": xsqT,
            "cc2_in": cc2,
            "ss2_in": ss2,
            "w1t_in": w1tile,
            "b1_in": np.ascontiguousarray(b1c.reshape(NBLK1, P)),
            "w2t_in": w2tile,
            "b2_in": np.ascontiguousarray(b2c.reshape(HC, P)),
            "qs_in": qsc,
            "ks_in": ksc,
        })
    return in_maps


def kernel(**inputs):
    if "nc" not in _CACHED:
        _CACHED["nc"] = _build_nc()
    nc = _CACHED["nc"]
    in_maps = _host_prep(inputs)
    res = run_bass_kernel_spmd(nc, in_maps, core_ids=list(range(NCORES)))
    acc = np.zeros((HID, L), dtype=np.float64)
    for c in range(NCORES):
        acc += res.results[c]["out_part"].reshape(HID, L)
    out = inputs["x"].astype(np.float32).reshape(L, HID) + acc.T.astype(np.float32)
    return out.reshape(1, L, HID).astype(np.float32)


# revision 37
# speedup vs baseline: 1.0704x; 1.0129x over previous
"""DreamFit single-stream processor block on 8 Trainium2 NeuronCores.

Sharding: 3 heads per core for SDPA (column-parallel linear1), 1/8 of the MLP
per core, row-parallel linear2 (host sums the 8 partial outputs).

Host-side folds (all cheap numpy): LoRA branches into w1/w2 (lora_weight==1),
the modulation (shift/scale/gate from vec) into w1 columns / b1 / w2 rows, so
the device kernel is just: LN-normalize -> linear1 -> qknorm/rope -> SDPA ->
gelu -> linear2 partial. x is passed pre-transposed; LN stats run as
bn_stats on the (otherwise idle) GpSimd engine from a row-major copy of x,
except the first quarter which uses PE column-sums (of x and a pre-squared
x slice) to keep the prologue short. All matmul operands are bf16 (full PE
rate, fast weight load, half the DMA); accumulation is fp32 in PSUM. V tiles
are transposed by the DMA xbar. The rope sign pattern and the QK-norm scales
are folded into precomputed cos/sin product tiles so the per-tile critical
path is two multiplies, a partition swap, and an add.
"""
import math

import numpy as np
import ml_dtypes
from contextlib import ExitStack

import concourse.bass as bass
import concourse.mybir as mybir
import concourse.tile as tile
from concourse import bacc
from concourse.bass_utils import run_bass_kernel_spmd
from concourse.masks import make_identity

F32 = mybir.dt.float32
F32R = mybir.dt.float32r
BF16 = mybir.dt.bfloat16
AF = mybir.ActivationFunctionType
ALU = mybir.AluOpType

P = 128
HID = 3072
HEADS = 24
HD = 128
MLP = 4 * HID            # 12288
L = 2048
NCORES = 8
H_PER = HEADS // NCORES  # 3 heads per core
DQK = H_PER * HD         # 384 q (and k, v) out dims per core
DMLP = MLP // NCORES     # 1536 mlp dims per core
DOUT1 = 3 * DQK + DMLP   # 2688 linear1 out dims per core
NBLK1 = DOUT1 // P       # 21 (0-2 q, 3-5 k, 6-8 v, 9-20 mlp)
NMLP = DMLP // P         # 12 mlp blocks
CATD = DQK + DMLP        # 1920 cat dims per core
NCAT = CATD // P         # 15
HC = HID // P            # 24 hidden chunks
NQ = 4                   # token quarters
LQ = L // NQ             # 512
LB = LQ // P             # 4 token tiles per quarter
NKB = L // P             # 16 key blocks
EPS = 1e-6

# linear1 block emission order: head-major (q0,k0,v0,...) so that attention
# head h's inputs complete as early as possible in the last quarter.
BLK_ORDER = [0, 3, 6, 1, 4, 7, 2, 5, 8] + list(range(9, NBLK1))

_CACHED = {}


def _build_nc():
    nc = bacc.Bacc("TRN2", target_bir_lowering=False, debug=False,
                   num_devices=NCORES)
    xT_in = nc.dram_tensor("xT_in", [HID, L], BF16, kind="ExternalInput")
    x_in = nc.dram_tensor("x_in", [L, HID], BF16, kind="ExternalInput")
    xsq0_in = nc.dram_tensor("xsq0_in", [HID, LQ], BF16, kind="ExternalInput")
    # cc2: [cos; cos] stacked; ssn: [sin; -sin] (sign of the rope cross term)
    cc2_in = nc.dram_tensor("cc2_in", [P, L], BF16, kind="ExternalInput")
    ssn_in = nc.dram_tensor("ssn_in", [P, L], BF16, kind="ExternalInput")
    w1t_in = nc.dram_tensor("w1t_in", [NBLK1, P, HC, P], BF16,
                            kind="ExternalInput")
    b1_in = nc.dram_tensor("b1_in", [NBLK1, P], F32, kind="ExternalInput")
    w2t_in = nc.dram_tensor("w2t_in", [HC, P, NCAT, P], BF16,
                            kind="ExternalInput")
    b2_in = nc.dram_tensor("b2_in", [HC, P], F32, kind="ExternalInput")
    qs_in = nc.dram_tensor("qs_in", [HD], F32, kind="ExternalInput")
    ks_in = nc.dram_tensor("ks_in", [HD], F32, kind="ExternalInput")
    out_t = nc.dram_tensor("out_part", [HC, P, L], F32, kind="ExternalOutput")

    with tile.TileContext(nc) as tc, \
            nc.allow_low_precision(reason="bf16 matmul pipeline is intentional"):
        _emit(nc, tc, xT_in, x_in, xsq0_in, cc2_in, ssn_in, w1t_in, b1_in,
              w2t_in, b2_in, qs_in, ks_in, out_t)
    nc.compile()
    return nc


def _emit(nc, tc, xT_in, x_in, xsq0_in, cc2_in, ssn_in, w1t_in, b1_in,
          w2t_in, b2_in, qs_in, ks_in, out_t):
    with ExitStack() as top:
        const = top.enter_context(tc.tile_pool(name="const", bufs=1))
        psum = top.enter_context(tc.tile_pool(name="psum", bufs=4, space="PSUM"))
        pscol = top.enter_context(tc.tile_pool(name="pscol", bufs=3, space="PSUM"))
        pstr = top.enter_context(tc.tile_pool(name="pstr", bufs=1, space="PSUM"))

        # ---- constants ----
        ident_f = const.tile([P, P], F32)
        make_identity(nc, ident_f)
        identb = const.tile([P, P], BF16)
        nc.vector.tensor_copy(identb, ident_f)
        ones_cb = const.tile([P, 1], BF16)       # K=128, M=1 column of ones
        nc.vector.memset(ones_cb, 1.0)
        ones_rb = const.tile([1, P], BF16)       # K=1, M=128 broadcast row
        nc.vector.memset(ones_rb, 1.0)
        eps_1 = const.tile([1, 1], F32)
        nc.vector.memset(eps_1, EPS)
        eps_c = const.tile([P, 1], F32)
        nc.vector.memset(eps_c, EPS)
        cc2 = const.tile([P, L], BF16)
        nc.sync.dma_start(out=cc2, in_=cc2_in[:, :])
        ssn = const.tile([P, L], BF16)
        nc.sync.dma_start(out=ssn, in_=ssn_in[:, :])
        qs = const.tile([P, 1], F32)
        nc.sync.dma_start(out=qs, in_=qs_in[:, None])
        ks = const.tile([P, 1], F32)
        nc.sync.dma_start(out=ks, in_=ks_in[:, None])
        b1t = const.tile([P, NBLK1], F32)
        nc.sync.dma_start(out=b1t, in_=b1_in.rearrange("b p -> p b"))
        b2t = const.tile([P, HC], F32)
        nc.sync.dma_start(out=b2t, in_=b2_in.rearrange("b p -> p b"))

        glp = top.enter_context(tc.tile_pool(name="glp", bufs=1))
        gelT = [glp.tile([P, L], BF16, tag=f"gl{i}", name=f"gl{i}")
                for i in range(NMLP)]
        atp = top.enter_context(tc.tile_pool(name="attn", bufs=1))
        aoT = [atp.tile([P, L], BF16, tag=f"ao{h}", name=f"ao{h}")
               for h in range(H_PER)]

        with ExitStack() as bc_scope:
            qkv = bc_scope.enter_context(tc.tile_pool(name="qkv", bufs=1))
            qkT = [qkv.tile([P, L], BF16, tag=f"q{h}", name=f"q{h}")
                   for h in range(H_PER)] + \
                  [qkv.tile([P, L], BF16, tag=f"k{h}", name=f"k{h}")
                   for h in range(H_PER)]
            vblk = [[None] * NKB for _ in range(H_PER)]

            # ========================================================
            # Phase B: per quarter: LN stats -> normalize -> linear1
            # ========================================================
            with ExitStack() as bb:
                xrp = bb.enter_context(tc.tile_pool(name="xr", bufs=2))
                xmp = bb.enter_context(tc.tile_pool(name="xrm", bufs=2))
                xsp = bb.enter_context(tc.tile_pool(name="xsq", bufs=6))
                w1p = bb.enter_context(tc.tile_pool(name="w1s", bufs=3))
                lnp = bb.enter_context(tc.tile_pool(name="ln", bufs=1))
                vqp = bb.enter_context(tc.tile_pool(name="vq", bufs=2))
                xT_r = xT_in.rearrange("(c p) l -> p c l", p=P)
                x_r = x_in.rearrange("(t p) h -> t p h", p=P)
                xsq0_r = xsq0_in.rearrange("(c p) l -> p c l", p=P)

                def prep0():
                    """Quarter 0: PE column-sum stats for a short prologue."""
                    xq = xrp.tile([P, HC, LQ], BF16, tag="xq")
                    for hc in range(HC):
                        nc.scalar.dma_start(out=xq[:, hc],
                                            in_=xT_r[:, hc, 0:LQ])
                    psx = pscol.tile([1, LQ], F32, tag="col")
                    for hc in range(HC):
                        nc.tensor.matmul(psx, ones_cb, xq[:, hc],
                                         start=(hc == 0), stop=(hc == HC - 1))
                    psq = pscol.tile([1, LQ], F32, tag="col")
                    for hc in range(HC):
                        xsq = xsp.tile([P, LQ], BF16, tag="xsq0")
                        nc.scalar.dma_start(out=xsq, in_=xsq0_r[:, hc, :])
                        nc.tensor.matmul(psq, ones_cb, xsq,
                                         start=(hc == 0), stop=(hc == HC - 1))
                    mu_row = lnp.tile([1, LQ], BF16, tag="mu")
                    nc.scalar.activation(mu_row, psx, AF.Copy, scale=1.0 / HID)
                    musq = lnp.tile([1, LQ], F32, tag="musq")
                    nc.scalar.square(musq, mu_row)
                    var = lnp.tile([1, LQ], F32, tag="var")
                    nc.vector.scalar_tensor_tensor(
                        var, psq, 1.0 / HID, musq, ALU.mult, ALU.subtract)
                    rinv = lnp.tile([1, LQ], BF16, tag="rinv")
                    nc.scalar.activation(rinv, var, AF.Abs_reciprocal_sqrt,
                                         bias=eps_1)
                    return xq, mu_row, rinv

                def prep_bn(q):
                    """Quarters 1-3: bn_stats on GpSimd; packs mu/rstd into
                    a [128, 8] tile, transposed to rows by the PE."""
                    qsl = slice(q * LQ, (q + 1) * LQ)
                    xq = xrp.tile([P, HC, LQ], BF16, tag="xq")
                    nc.scalar.dma_start(out=xq, in_=xT_r[:, :, qsl])
                    stg = lnp.tile([P, 8], BF16, tag="stg")
                    for t in range(LB):
                        xm = xmp.tile([P, HID], BF16, tag="xm")
                        nc.scalar.dma_start(out=xm, in_=x_r[q * LB + t])
                        stats = xmp.tile([P, 6, 6], F32, tag="stats")
                        for sgp in range(6):
                            nc.vector.bn_stats(
                                out=stats[:, sgp, :],
                                in_=xm[:, sgp * 512:(sgp + 1) * 512])
                        mv = xmp.tile([P, 2], F32, tag="mv")
                        nc.vector.bn_aggr(out=mv, in_=stats)
                        nc.vector.tensor_copy(stg[:, t:t + 1], mv[:, 0:1])
                        rst = xmp.tile([P, 1], F32, tag="rst")
                        nc.scalar.activation(rst, mv[:, 1:2],
                                             AF.Abs_reciprocal_sqrt,
                                             bias=eps_c)
                        nc.vector.tensor_copy(stg[:, 4 + t:5 + t], rst)
                    return xq, stg

                def norm_bn(state, q):
                    xq, stg = state
                    # transpose [128, 8] -> [8, 128]: rows 0-3 mu, 4-7 rstd
                    pst = pstr.tile([8, P], BF16, tag="trp")
                    nc.tensor.transpose(pst, stg, identb)
                    st8 = lnp.tile([8, P], BF16, tag="st8")
                    nc.scalar.copy(st8, pst)
                    mu_row = lnp.tile([1, LQ], BF16, tag="mu")
                    rinv = lnp.tile([1, LQ], BF16, tag="rinv")
                    for t in range(LB):
                        nc.sync.dma_start(out=mu_row[:, t * P:(t + 1) * P],
                                          in_=st8[t:t + 1, :])
                        nc.sync.dma_start(out=rinv[:, t * P:(t + 1) * P],
                                          in_=st8[4 + t:5 + t, :])
                    return norm_apply((xq, mu_row, rinv))

                def norm_apply(state):
                    """Broadcast the stats and normalize x^T in place."""
                    xq, mu_row, rinv = state
                    pmu = psum.tile([P, LQ], F32, tag="big")
                    nc.tensor.matmul(pmu, ones_rb, mu_row, start=True, stop=True)
                    mu_b = lnp.tile([P, LQ], BF16, tag="mub")
                    nc.scalar.copy(mu_b, pmu)
                    prs = psum.tile([P, LQ], F32, tag="big")
                    nc.tensor.matmul(prs, ones_rb, rinv, start=True, stop=True)
                    r_b = lnp.tile([P, LQ], BF16, tag="rb")
                    nc.scalar.copy(r_b, prs)
                    # normalize in place: xq <- (xq - mu) * rstd
                    for hc in range(HC):
                        nc.vector.tensor_sub(xq[:, hc], xq[:, hc], mu_b)
                        nc.vector.tensor_mul(xq[:, hc], xq[:, hc], r_b)
                    return xq

                def lin1_block(q, xq, blk):
                    qsl = slice(q * LQ, (q + 1) * LQ)
                    w1t = w1p.tile([P, HC, P], BF16, tag="w1t")
                    nc.sync.dma_start(out=w1t, in_=w1t_in[blk])
                    ps = psum.tile([P, LQ], F32, tag="big")
                    for hc in range(HC):
                        nc.tensor.matmul(ps, w1t[:, hc], xq[:, hc],
                                         start=(hc == 0), stop=(hc == HC - 1))
                    if blk < 6:       # q / k -> qkT slices
                        nc.vector.tensor_scalar_add(
                            qkT[blk][:, qsl], ps, b1t[:, blk:blk + 1])
                    elif blk < 9:     # v: evict, then DMA-xbar transpose
                        h = blk - 6
                        vq = vqp.tile([P, LQ], BF16, tag="vq")
                        nc.vector.tensor_scalar_add(vq, ps,
                                                    b1t[:, blk:blk + 1])
                        for j in range(LB):
                            vb = qkv.tile([P, P], BF16,
                                          tag=f"vb{h}_{q * LB + j}",
                                          name=f"vb{h}_{q * LB + j}")
                            nc.scalar.dma_start_transpose(
                                vb, vq[:, j * P:(j + 1) * P])
                            vblk[h][q * LB + j] = vb
                    else:             # mlp -> gelu, resident in SBUF
                        nc.scalar.activation(gelT[blk - 9][:, qsl], ps,
                                             AF.Gelu_apprx_tanh,
                                             bias=b1t[:, blk:blk + 1])

                # software-pipelined: quarter q+1's stats and normalize are
                # emitted a few blocks into quarter q's linear1, so the
                # x DMA and stats chain hide under the matmul stream.
                xq_prev = norm_apply(prep0())
                for q in range(NQ):
                    for blk in BLK_ORDER[:3]:
                        lin1_block(q, xq_prev, blk)
                    if q + 1 < NQ:
                        st_next = prep_bn(q + 1)
                    for blk in BLK_ORDER[3:9]:
                        lin1_block(q, xq_prev, blk)
                    if q + 1 < NQ:
                        xq_next = norm_bn(st_next, q + 1)
                    for blk in BLK_ORDER[9:]:
                        lin1_block(q, xq_prev, blk)
                    if q + 1 < NQ:
                        xq_prev = xq_next

            # ========================================================
            # Phase C: QK-norm (RMS over head dim, on partitions) + rope,
            # emitted stage-major so per-tile chains pipeline across tiles.
            # All Abs_reciprocal_sqrt activations run before attention so
            # the scalar engine's table never swaps during the exp stream.
            # ========================================================
            with ExitStack() as cc:
                rmsp = cc.enter_context(tc.tile_pool(name="rms", bufs=2))
                sqp = cc.enter_context(tc.tile_pool(name="sqp", bufs=3))
                srp = cc.enter_context(tc.tile_pool(name="srp", bufs=16))
                rtp = cc.enter_context(tc.tile_pool(name="rtp", bufs=3))
                ptp = cc.enter_context(tc.tile_pool(name="ptp", bufs=34))
                sdp = cc.enter_context(tc.tile_pool(name="sdp", bufs=2))

                # C tiles in the order attention consumes them: q0,k0,q1,...
                CT = []
                for h in range(H_PER):
                    CT.append((qkT[h], qs))
                    CT.append((qkT[H_PER + h], ks))
                cst = [{} for _ in range(6)]

                def c_sq(i):
                    t, _ = CT[i]
                    sq = sqp.tile([P, L], BF16, tag="sq")
                    nc.vector.tensor_mul(sq, t, t)
                    cst[i]["sq"] = sq

                def c_sums(i):
                    rinvs = []
                    for j in range(NQ):
                        jsl = slice(j * LQ, (j + 1) * LQ)
                        pc = pscol.tile([1, LQ], F32, tag="col")
                        nc.tensor.matmul(pc, ones_cb, cst[i]["sq"][:, jsl],
                                         start=True, stop=True)
                        rinv = srp.tile([1, LQ], BF16, tag="rinv")
                        nc.scalar.activation(rinv, pc, AF.Abs_reciprocal_sqrt,
                                             bias=eps_1, scale=1.0 / HD)
                        rinvs.append(rinv)
                    cst[i]["rinvs"] = rinvs

                def c_bcast(i):
                    # Wc = (rstd*scale)*cos, Ws = (rstd*scale)*sin_signed,
                    # built straight from the broadcast PSUM
                    _, scale_ap = CT[i]
                    wc = rmsp.tile([P, L], BF16, tag="wc")
                    ws = rmsp.tile([P, L], BF16, tag="ws")
                    for j, rinv in enumerate(cst[i]["rinvs"]):
                        jsl = slice(j * LQ, (j + 1) * LQ)
                        pb = psum.tile([P, LQ], F32, tag="big")
                        nc.tensor.matmul(pb, ones_rb, rinv,
                                         start=True, stop=True)
                        nc.vector.scalar_tensor_tensor(
                            wc[:, jsl], pb, scale_ap, cc2[:, jsl],
                            ALU.mult, ALU.mult)
                        nc.vector.scalar_tensor_tensor(
                            ws[:, jsl], pb, scale_ap, ssn[:, jsl],
                            ALU.mult, ALU.mult)
                    cst[i]["wc"] = wc
                    cst[i]["ws"] = ws

                def c_finish(i):
                    t, _ = CT[i]
                    A = rtp.tile([P, L], BF16, tag="rt")   # t * rstd*s*cos
                    B = rtp.tile([P, L], BF16, tag="rt")   # t * rstd*s*sin+-
                    Bx = rtp.tile([P, L], BF16, tag="rt")  # halves swapped
                    nc.vector.tensor_mul(A, t, cst[i]["wc"])
                    nc.vector.tensor_mul(B, t, cst[i]["ws"])
                    nc.scalar.dma_start(out=Bx[0:64, :], in_=B[64:128, :])
                    nc.scalar.dma_start(out=Bx[64:128, :], in_=B[0:64, :])
                    nc.vector.tensor_add(t, A, Bx)

                # squares + row-sums + rsqrt for all six tiles, pipelined
                c_sq(0), c_sq(1)
                c_sums(0)
                c_sq(2)
                c_sums(1)
                c_sq(3)
                c_sums(2)
                c_sq(4)
                c_sums(3)
                c_sq(5)
                c_sums(4), c_sums(5)
                c_bcast(0), c_bcast(1)
                c_finish(0), c_finish(1)
                # tiles 2..5 are finished inside the attention loop below,
                # hiding their chains under head h-1's matmuls

                # ====================================================
                # Phase E: attention per head, one-quarter lookahead so
                # the Exp (scalar engine) hides under PE work
                # ====================================================
                def scores(h, qc):
                    qT, kT = qkT[h], qkT[H_PER + h]
                    qslc = slice(qc * LQ, (qc + 1) * LQ)
                    pts = []
                    for kb in range(NKB):
                        ps = psum.tile([P, LQ], F32, tag="big")
                        nc.tensor.matmul(ps, kT[:, kb * P:(kb + 1) * P],
                                         qT[:, qslc], start=True, stop=True)
                        ptile = ptp.tile([P, LQ], BF16, tag="pt", name="pt")
                        nc.scalar.activation(ptile, ps, AF.Exp)
                        pts.append(ptile)
                    return pts

                def finish(h, qc, pts):
                    qslc = slice(qc * LQ, (qc + 1) * LQ)
                    psd = pscol.tile([1, LQ], F32, tag="col")
                    for kb in range(NKB):
                        nc.tensor.matmul(psd, ones_cb, pts[kb],
                                         start=(kb == 0), stop=(kb == NKB - 1))
                    rdf = sdp.tile([1, LQ], F32, tag="rdf")
                    nc.vector.reciprocal_approx_fast(rdf, psd)
                    rd = sdp.tile([1, LQ], BF16, tag="rd")
                    nc.scalar.copy(rd, rdf)
                    # attn @ V accumulation first: it does not depend on the
                    # reciprocal chain, so the PE never waits for it
                    pso = psum.tile([P, LQ], F32, tag="big")
                    for kb in range(NKB):
                        nc.tensor.matmul(pso, vblk[h][kb], pts[kb],
                                         start=(kb == 0), stop=(kb == NKB - 1))
                    pbd = psum.tile([P, LQ], F32, tag="big")
                    nc.tensor.matmul(pbd, ones_rb, rd, start=True, stop=True)
                    rbd = sdp.tile([P, LQ], BF16, tag="rbd")
                    nc.scalar.copy(rbd, pbd)
                    nc.vector.tensor_mul(aoT[h][:, qslc], pso, rbd)

                prev = None
                for h in range(H_PER):
                    for qc in range(NQ):
                        pts = scores(h, qc)
                        if qc == 0 and h + 1 < H_PER:
                            # prepare head h+1's q/k while head h runs
                            c_bcast(2 * h + 2), c_bcast(2 * h + 3)
                        elif qc == 1 and h + 1 < H_PER:
                            c_finish(2 * h + 2), c_finish(2 * h + 3)
                        if prev is not None:
                            finish(*prev)
                        prev = (h, qc, pts)
                finish(*prev)

        # ========================================================
        # Phase F: linear2 (row-parallel partial); +b2 on core 0
        # ========================================================
        with ExitStack() as fc:
            w2p = fc.enter_context(tc.tile_pool(name="w2p", bufs=3))
            otp = fc.enter_context(tc.tile_pool(name="otp", bufs=4))
            catT = aoT + gelT  # 15 chunks of [128, L]
            for blk in range(HC):
                w2t = w2p.tile([P, NCAT, P], BF16, tag="w2t")
                nc.sync.dma_start(out=w2t, in_=w2t_in[blk])
                for lc in range(NQ):
                    lsl = slice(lc * LQ, (lc + 1) * LQ)
                    ps = psum.tile([P, LQ], F32, tag="big")
                    for cat_i in range(NCAT):
                        nc.tensor.matmul(ps, w2t[:, cat_i], catT[cat_i][:, lsl],
                                         start=(cat_i == 0),
                                         stop=(cat_i == NCAT - 1))
                    ot = otp.tile([P, LQ], F32, tag="ot")
                    nc.vector.tensor_scalar_add(ot, ps, b2t[:, blk:blk + 1])
                    nc.scalar.dma_start(out=out_t[blk, :, lsl], in_=ot)


def _bf16(a):
    return np.ascontiguousarray(a.astype(ml_dtypes.bfloat16))


def _host_prep(inputs):
    f = np.float32
    perm = np.concatenate([np.arange(0, HD, 2), np.arange(1, HD, 2)])
    # ---- LoRA folds ----
    w1_eff = inputs["w1"].astype(f).copy()
    for i, nm in enumerate(("q", "k", "v")):
        up = inputs[f"lora_{nm}_up"].astype(f)
        dn = inputs[f"lora_{nm}_down"].astype(f)
        w1_eff[i * HID:(i + 1) * HID] += up @ dn
    w2_eff = inputs["w2"].astype(f) + \
        inputs["proj_up"].astype(f) @ inputs["proj_down"].astype(f)
    # ---- modulation fold (shift/scale/gate from vec) ----
    vec = inputs["vec"].astype(np.float64).reshape(HID)
    sv = (vec / (1.0 + np.exp(-vec)))
    m = sv @ inputs["mod_w"].astype(np.float64).T + \
        inputs["mod_b"].astype(np.float64)
    shift, scale, gate = np.split(m.astype(f), 3)
    b1_eff = inputs["b1"].astype(f) + w1_eff @ shift
    w1s = w1_eff * (1.0 + scale)[None, :]
    w2g = w2_eff * gate[:, None]
    b2g = inputs["b2"].astype(f) * gate

    wq = w1s[0:HID].reshape(HEADS, HD, HID)[:, perm, :]
    wk = w1s[HID:2 * HID].reshape(HEADS, HD, HID)[:, perm, :]
    wv = w1s[2 * HID:3 * HID].reshape(HEADS, HD, HID)
    wm = w1s[3 * HID:].reshape(NCORES, DMLP, HID)
    bq = b1_eff[0:HID].reshape(HEADS, HD)[:, perm]
    bk = b1_eff[HID:2 * HID].reshape(HEADS, HD)[:, perm]
    bv = b1_eff[2 * HID:3 * HID].reshape(HEADS, HD)
    bm = b1_eff[3 * HID:].reshape(NCORES, DMLP)

    pe = inputs["pe"].astype(f)
    cos = pe[0, 0, :, :, 0, 0]   # (L, 64)
    sin = pe[0, 0, :, :, 1, 0]   # (L, 64)
    cc2 = _bf16(np.concatenate([cos.T, cos.T], axis=0))    # (128, L)
    ssn = _bf16(np.concatenate([sin.T, -sin.T], axis=0))   # (128, L)

    qsc = inputs["q_scale"].astype(f)[perm] / math.sqrt(HD)
    ksc = inputs["k_scale"].astype(f)[perm]
    x2d = inputs["x"].astype(f).reshape(L, HID)
    xb = x2d.astype(ml_dtypes.bfloat16)
    xT = np.ascontiguousarray(xb.T)
    xsq0 = np.ascontiguousarray(
        (xb[0:LQ].astype(f) ** 2).astype(ml_dtypes.bfloat16).T)

    in_maps = []
    for c in range(NCORES):
        hs = slice(H_PER * c, H_PER * (c + 1))
        w1c = np.concatenate([
            wq[hs].reshape(DQK, HID), wk[hs].reshape(DQK, HID),
            wv[hs].reshape(DQK, HID), wm[c]], axis=0)      # (2688, 3072)
        b1c = np.concatenate([
            bq[hs].reshape(DQK), bk[hs].reshape(DQK), bv[hs].reshape(DQK),
            bm[c]])
        w2c = np.concatenate([
            w2g[:, DQK * c:DQK * (c + 1)],
            w2g[:, HID + DMLP * c:HID + DMLP * (c + 1)]], axis=1)  # (3072,1920)
        b2c = b2g if c == 0 else np.zeros_like(b2g)
        w1tile = _bf16(w1c.reshape(NBLK1, P, HC, P).transpose(0, 3, 2, 1))
        w2tile = _bf16(w2c.reshape(HC, P, NCAT, P).transpose(0, 3, 2, 1))
        in_maps.append({
            "xT_in": xT,
            "x_in": xb,
            "xsq0_in": xsq0,
            "cc2_in": cc2,
            "ssn_in": ssn,
            "w1t_in": w1tile,
            "b1_in": np.ascontiguousarray(b1c.reshape(NBLK1, P)),
            "w2t_in": w2tile,
            "b2_in": np.ascontiguousarray(b2c.reshape(HC, P)),
            "qs_in": qsc,
            "ks_in": ksc,
        })
    return in_maps


def kernel(**inputs):
    if "nc" not in _CACHED:
        _CACHED["nc"] = _build_nc()
    nc = _CACHED["nc"]
    in_maps = _host_prep(inputs)
    res = run_bass_kernel_spmd(nc, in_maps, core_ids=list(range(NCORES)))
    acc = np.zeros((HID, L), dtype=np.float64)
    for c in range(NCORES):
        acc += res.results[c]["out_part"].reshape(HID, L)
    out = inputs["x"].astype(np.float32).reshape(L, HID) + acc.T.astype(np.float32)
    return out.reshape(1, L, HID).astype(np.float32)
